# revision 1
# baseline (speedup 1.0000x reference)
"""Multi-head cross-attention Trainium2 kernel (8 NeuronCores, SPMD).

Problem: nn_MultiHeadCrossAttention_31791347925263
  x:[4,2048,768], y:[4,2048,768], 12 heads x 64, fp32.
  out = softmax((x Wq^T)(y Wk^T)^T / 8 + mask) (y Wv^T) Wo^T   (+ zero biases)

Sharding: 8 cores = (batch b in 0..3) x (query half in 0..1). Each core
computes the full attention for its 1024 query rows against all 2048 keys
of its batch. No collectives; outputs concatenate.

Per-core dataflow (all matmuls in float32r = TF32-like, 11-bit mantissa RNE):
  host:  xT=[768,1024], yT=[768,2048], WqT/WkT/WvT/WoT = W.T contiguous
         (k/v rows of Wkv are interleaved per head: 64 k then 64 v per 128)
  kT  = WkT-blocks^T-matmul yT      -> [768(k-dim), 2048(sk)]
  v'  = yT-blocks^T-matmul WvT      -> [2048(sk), 780] (65 cols/head: v|ones)
  qT  = WqT-blocks^T-matmul xT      -> [768(q-dim), 1024(sq)]
  per head pair (2*hb, 2*hb+1), per sk-block (128 keys):
      S^T = kT_h-block^T-matmul qT_h      -> PSUM [128, 1024] (row groups
                                             0-63/64-127 alternate -> the two
                                             heads' K=64 matmuls overlap)
      P~  = exp(S^T * 0.125)              -> SBUF f32r (ACT, no max-subtract:
                                             scores ~ N(0,1), max ~ 4)
      valT_h += v'[skb,h]^T-matmul P~     -> PSUM [65, 1024]
                                             (row 64 = softmax denominator)
  valnorm_h = valT_h[0:64] * bcast(1/valT_h[64])
      (DVE copy + fast-reciprocal, GPSIMD partition-broadcast; valnorm
       overwrites qT's tiles - same [128,1024] f32r shape, qT[hb] is dead
       once pair hb's QK matmuls are done)
  o[sqb]    = valnorm-blocks^T-matmul WoT -> [1024, 768] -> DMA out

All matmul outputs except valT share one 2-buf [128,1024] PSUM tag
(2 banks per slot; valT pool 2x2 banks) = exactly the 8 PSUM banks.
"""

import numpy as np

B, S, D = 4, 2048, 768
H, Dh = 12, 64
SQ = S // 2          # queries per core
N_CORES = 8
DB = D // 128        # 6 d_model blocks
SKB = S // 128       # 16 key blocks
SQB = SQ // 128      # 8 query blocks per core
VPW = H * (Dh + 1)   # 780: v' width (64 v cols + 1 ones col per head)

_cache = {}


def _build_nc():
    import concourse.mybir as mybir
    import concourse.tile as tile
    from concourse import bacc

    f32 = mybir.dt.float32
    f32r = mybir.dt.float32r
    EXP = mybir.ActivationFunctionType.Exp

    nc = bacc.Bacc("TRN2", target_bir_lowering=False)
    xT = nc.dram_tensor("xT", [D, SQ], f32, kind="ExternalInput")
    yT = nc.dram_tensor("yT", [D, S], f32, kind="ExternalInput")
    WqT = nc.dram_tensor("WqT", [D, D], f32, kind="ExternalInput")
    WkT = nc.dram_tensor("WkT", [D, D], f32, kind="ExternalInput")
    WvT = nc.dram_tensor("WvT", [D, D], f32, kind="ExternalInput")
    WoT = nc.dram_tensor("WoT", [D, D], f32, kind="ExternalInput")
    out = nc.dram_tensor("out", [SQ, D], f32, kind="ExternalOutput")

    with tile.TileContext(nc) as tc:
        with tc.tile_pool(name="persist", bufs=1) as pp, \
             tc.tile_pool(name="mmps", bufs=2, space="PSUM") as mm_ps, \
             tc.tile_pool(name="vtp", bufs=2, space="PSUM") as vt_ps:

            def mm_tile(cols):
                return mm_ps.tile([128, cols], f32, name="mmps", tag="mmps",
                                  padded_shape=[128, SQ])

            kT = [pp.tile([128, S], f32r, name=f"kT{i}") for i in range(DB)]
            vp = [pp.tile([128, VPW], f32r, name=f"vp{i}") for i in range(SKB)]
            qT = [pp.tile([128, SQ], f32r, name=f"qT{i}") for i in range(DB)]
            vnorm = qT  # valnorm overwrites qT (same shape; see docstring)

            with tc.tile_pool(name="ld_y", bufs=1) as ld_y:
                yTs = [ld_y.tile([128, S], f32r, name=f"yTs{i}")
                       for i in range(DB)]

                # ---- kT projection: kT[ob] = (WkT col-block)^T @ yT ----
                with tc.tile_pool(name="ld_wk", bufs=1) as ld_wk:
                    wkTs = [ld_wk.tile([128, D], f32r, name=f"wkTs{i}")
                            for i in range(DB)]
                    for i in range(DB):
                        nc.sync.dma_start(
                            out=wkTs[i],
                            in_=WkT[i * 128:(i + 1) * 128, :].bitcast(f32r))
                    for c4 in range(4):
                        for i in range(DB):
                            nc.sync.dma_start(
                                out=yTs[i][:, c4 * 512:(c4 + 1) * 512],
                                in_=yT[i * 128:(i + 1) * 128,
                                       c4 * 512:(c4 + 1) * 512].bitcast(f32r))
                    wvTs = [ld_y.tile([128, D], f32r, name=f"wvTs{i}")
                            for i in range(DB)]
                    for i in range(DB):
                        nc.sync.dma_start(
                            out=wvTs[i],
                            in_=WvT[i * 128:(i + 1) * 128, :].bitcast(f32r))
                    # nc4 outer: the first 6 groups need only yT column
                    # chunk 0, so compute starts while chunks 1-3 stream in
                    for nc4 in range(4):
                        for ob in range(DB):
                            ps = mm_tile(512)
                            for kb in range(DB):
                                nc.tensor.matmul(
                                    ps[:, :],
                                    wkTs[kb][:, ob * 128:(ob + 1) * 128],
                                    yTs[kb][:, nc4 * 512:(nc4 + 1) * 512],
                                    start=(kb == 0), stop=(kb == DB - 1))
                            nc.vector.tensor_copy(
                                kT[ob][:, nc4 * 512:(nc4 + 1) * 512], ps[:, :])

                # ---- v' projection: v[skb] = (yT blk)^T @ WvT ----
                if True:
                    for skb in range(SKB):
                        vps3 = vp[skb].rearrange("p (h c) -> p h c", c=Dh + 1)
                        nc.vector.memset(vps3[:, :, Dh].bitcast(f32), 1.0)
                        for nc2 in range(2):
                            n0, n1 = nc2 * 512, min(D, (nc2 + 1) * 512)
                            ps = mm_tile(512)
                            for kb in range(DB):
                                nc.tensor.matmul(
                                    ps[:, 0:n1 - n0],
                                    yTs[kb][:, skb * 128:(skb + 1) * 128],
                                    wvTs[kb][:, n0:n1],
                                    start=(kb == 0), stop=(kb == DB - 1))
                            # contiguous v-cols -> 65-strided layout
                            src = ps[:, 0:n1 - n0].rearrange(
                                "p (h c) -> p h c", c=Dh)
                            dst = vps3[:, nc2 * 8:nc2 * 8 + (n1 - n0) // Dh,
                                       0:Dh]
                            nc.vector.tensor_copy(dst, src)

            # ---- qT projection ----
            with tc.tile_pool(name="ld_x", bufs=1) as ld_x:
                xTs = [ld_x.tile([128, SQ], f32r, name=f"xTs{i}")
                       for i in range(DB)]
                wqTs = [ld_x.tile([128, D], f32r, name=f"wqTs{i}")
                        for i in range(DB)]
                # critical-path order: weights, then xT halves in chunk order
                for i in range(DB):
                    nc.sync.dma_start(
                        out=wqTs[i],
                        in_=WqT[i * 128:(i + 1) * 128, :].bitcast(f32r))
                for c2 in range(2):
                    for i in range(DB):
                        nc.sync.dma_start(
                            out=xTs[i][:, c2 * 512:(c2 + 1) * 512],
                            in_=xT[i * 128:(i + 1) * 128,
                                   c2 * 512:(c2 + 1) * 512].bitcast(f32r))
                for nc2 in range(2):
                    for ob in range(DB):
                        ps = mm_tile(512)
                        for kb in range(DB):
                            nc.tensor.matmul(
                                ps[:, :],
                                wqTs[kb][:, ob * 128:(ob + 1) * 128],
                                xTs[kb][:, nc2 * 512:(nc2 + 1) * 512],
                                start=(kb == 0), stop=(kb == DB - 1))
                        nc.vector.tensor_copy(
                            qT[ob][:, nc2 * 512:(nc2 + 1) * 512], ps[:, :])

            # ---- attention ----
            with tc.tile_pool(name="late", bufs=1) as lp:
                woT = [lp.tile([128, D], f32r, name=f"woT{i}")
                       for i in range(DB)]
                for i in range(DB):
                    nc.sync.dma_start(
                        out=woT[i],
                        in_=WoT[i * 128:(i + 1) * 128, :].bitcast(f32r))

                with tc.tile_pool(name="psb", bufs=5) as p_pool, \
                     tc.tile_pool(name="nrm", bufs=2) as nrm_pool:
                    for hb in range(H // 2):
                        h0, h1 = 2 * hb, 2 * hb + 1
                        vt0 = vt_ps.tile([65, SQ], f32, name="valT")
                        vt1 = vt_ps.tile([65, SQ], f32, name="valT")
                        for skb in range(SKB):
                            st0 = mm_tile(SQ)
                            st1 = mm_tile(SQ)
                            for j in range(2):
                                for r0, st in ((0, st0), (64, st1)):
                                    nc.tensor.matmul(
                                        st[:, j * 512:(j + 1) * 512],
                                        kT[hb][r0:r0 + 64,
                                               skb * 128:(skb + 1) * 128],
                                        qT[hb][r0:r0 + 64,
                                               j * 512:(j + 1) * 512],
                                        start=True, stop=True)
                            pt0 = p_pool.tile([128, SQ], f32r, name="pT")
                            pt1 = p_pool.tile([128, SQ], f32r, name="pT")
                            nc.scalar.activation(pt0[:, :], st0[:, :], EXP,
                                                 scale=0.125)
                            nc.scalar.activation(pt1[:, :], st1[:, :], EXP,
                                                 scale=0.125)
                            for h, vt, pt in ((h0, vt0, pt0), (h1, vt1, pt1)):
                                for j in range(2):
                                    nc.tensor.matmul(
                                        vt[:, j * 512:(j + 1) * 512],
                                        vp[skb][:, h * 65:h * 65 + 65],
                                        pt[:, j * 512:(j + 1) * 512],
                                        start=(skb == 0),
                                        stop=(skb == SKB - 1))
                        for h, vt in ((h0, vt0), (h1, vt1)):
                            r0 = (h % 2) * 64
                            # single fast copy frees the PSUM accumulator so
                            # the next pair's PV can start immediately
                            vals = nrm_pool.tile([65, SQ], f32, name="vals")
                            nc.vector.tensor_copy(vals[:, :], vt[:, :])
                            rec = nrm_pool.tile([1, SQ], f32, name="rec")
                            nc.vector.reciprocal(rec[:, :], vals[64:65, :])
                            rbc = nrm_pool.tile([64, SQ], f32, name="rbc")
                            nc.gpsimd.partition_broadcast(rbc[:, :], rec[:, :])
                            nc.vector.tensor_mul(
                                vnorm[hb][r0:r0 + 64, :], vals[0:64, :],
                                rbc[:, :])

                # ---- output projection ----
                # alternate PSUM slots between the mm pool and the (now idle)
                # valT pool -> 4 concurrent accumulation groups instead of 2
                with tc.tile_pool(name="osb", bufs=3) as o_pool:
                    for sqb in range(SQB):
                        if sqb % 2 == 0:
                            op = mm_tile(D)
                        else:
                            op = vt_ps.tile([128, D], f32, name="valT",
                                            tag="valT",
                                            padded_shape=[128, SQ])
                        for nc2 in range(2):
                            n0, n1 = nc2 * 512, min(D, (nc2 + 1) * 512)
                            for kb in range(DB):
                                nc.tensor.matmul(
                                    op[:, n0:n1],
                                    vnorm[kb][:, sqb * 128:(sqb + 1) * 128],
                                    woT[kb][:, n0:n1],
                                    start=(kb == 0), stop=(kb == DB - 1))
                        ot = o_pool.tile([128, D], f32, name="osb")
                        nc.vector.tensor_copy(ot[:, :], op[:, :])
                        nc.sync.dma_start(
                            out=out[sqb * 128:(sqb + 1) * 128, :], in_=ot[:, :])

    nc.compile()
    return nc


def _get_nc():
    if "nc" not in _cache:
        _cache["nc"] = _build_nc()
    return _cache["nc"]


def _host_fallback(x, y, mask, Wq, bq, Wkv, bkv, Wo, bo):
    Bb, Ss, _ = x.shape
    q = x @ Wq.T + bq
    kv = y @ Wkv.T + bkv
    q = q.reshape(Bb, Ss, H, Dh).transpose(0, 2, 1, 3)
    kv = kv.reshape(Bb, Ss, H, 2 * Dh).transpose(0, 2, 1, 3)
    k, v = kv[..., :Dh], kv[..., Dh:]
    scaled = np.einsum("bhqd,bhkd->bhqk", q, k) / np.sqrt(np.float32(Dh))
    scaled = scaled + mask
    scaled -= scaled.max(axis=-1, keepdims=True)
    e = np.exp(scaled)
    attn = e / e.sum(axis=-1, keepdims=True)
    values = np.einsum("bhqk,bhkd->bhqd", attn, v)
    values = values.transpose(0, 2, 1, 3).reshape(Bb, Ss, H * Dh)
    return (values @ Wo.T + bo).astype(np.float32)


def _run(inputs, trace=False, trace_cores=None):
    """Returns (full_output, BassKernelResults)."""
    from concourse.bass_utils import run_bass_kernel_spmd

    x = np.ascontiguousarray(np.asarray(inputs["x"], dtype=np.float32))
    y = np.ascontiguousarray(np.asarray(inputs["y"], dtype=np.float32))
    Wq = np.asarray(inputs["Wq"], dtype=np.float32)
    Wkv = np.asarray(inputs["Wkv"], dtype=np.float32)
    Wo = np.asarray(inputs["Wo"], dtype=np.float32)

    # Reference reshapes kv to [B,S,H,2*Dh]: per head, rows h*128..h*128+63 of
    # Wkv are the k-projection, rows h*128+64..h*128+127 the v-projection.
    k_rows = np.concatenate([np.arange(h * 128, h * 128 + Dh) for h in range(H)])
    v_rows = np.concatenate([np.arange(h * 128 + Dh, (h + 1) * 128)
                             for h in range(H)])
    WqT = np.ascontiguousarray(Wq.T)
    WkT = np.ascontiguousarray(Wkv[k_rows].T)
    WvT = np.ascontiguousarray(Wkv[v_rows].T)
    WoT = np.ascontiguousarray(Wo.T)

    in_maps = []
    for c in range(N_CORES):
        b, half = c // 2, c % 2
        xTc = np.ascontiguousarray(x[b, half * SQ:(half + 1) * SQ, :].T)
        yTb = np.ascontiguousarray(y[b].T)
        in_maps.append({"xT": xTc, "yT": yTb, "WqT": WqT, "WkT": WkT,
                        "WvT": WvT, "WoT": WoT})

    nc = _get_nc()
    res = run_bass_kernel_spmd(nc, in_maps, core_ids=list(range(N_CORES)),
                               trace=trace, trace_cores=trace_cores)
    out = np.empty((B, S, D), dtype=np.float32)
    for c in range(N_CORES):
        b, half = c // 2, c % 2
        out[b, half * SQ:(half + 1) * SQ, :] = res.results[c]["out"]
    return out, res


def kernel(**inputs) -> np.ndarray:
    mask = np.asarray(inputs["mask"], dtype=np.float32)
    bq = np.asarray(inputs["bq"], dtype=np.float32)
    bkv = np.asarray(inputs["bkv"], dtype=np.float32)
    bo = np.asarray(inputs["bo"], dtype=np.float32)
    if mask.any() or bq.any() or bkv.any() or bo.any():
        # Device kernel hardcodes zero mask/biases; stay correct regardless.
        return _host_fallback(
            np.asarray(inputs["x"], dtype=np.float32),
            np.asarray(inputs["y"], dtype=np.float32),
            mask, np.asarray(inputs["Wq"], dtype=np.float32), bq,
            np.asarray(inputs["Wkv"], dtype=np.float32), bkv,
            np.asarray(inputs["Wo"], dtype=np.float32), bo)
    out, _ = _run(inputs)
    return out

